# revision 5
# baseline (speedup 1.0000x reference)
"""Trainium2 Bass kernel for nn_ASD: LSTMCell over T=512 steps + linear readout.

Sharding: the 4096 gate columns (= 1024 hidden units x 4 gates) are sharded
8 ways; core p owns hidden units [128p, 128(p+1)) and computes their i/f/o/g
gates, cell state and hidden state. Each step the 8 hidden-state shards are
exchanged (AllGather) so every core has the full h_t for the next step's
recurrent matmul. Everything on-chip runs "transposed": hidden/gate units on
SBUF partitions, batch on the free dim, so no per-step transposes are needed
and elementwise ops use all 128 lanes.

Phase A precomputes P = x @ W_ih.T (+0) for this core's gate columns at full
PE utilization (moving free dim 512); phase B consumes P per step, adds the
recurrent term h_{t-1} @ W_hh.T via PSUM accumulation, applies the LSTM cell,
exchanges h shards, and accumulates the tiny readout matmul.

Compute dtype bf16 (PSUM accumulation fp32, cell state fp32).
"""
import sys
import numpy as np
import ml_dtypes

for _p in ("/opt/trn_rl_repo", "/root/.axon_site/_ro/trn_rl_repo"):
    if _p not in sys.path:
        sys.path.append(_p)

import concourse.bass as bass
import concourse.bacc as bacc
import concourse.mybir as mybir
import concourse.tile as tile
from concourse.bass_utils import run_bass_kernel_spmd

BF16 = ml_dtypes.bfloat16
N_CORES = 8
B = 32          # batch
F = 2048        # input features
H = 1024        # hidden
T_FULL = 512    # sequence length
KF = F // 128   # 16 feature chunks
KH = H // 128   # 8 hidden chunks
GRP = 16        # timesteps per projection group
FP32 = mybir.dt.float32
DBF16 = mybir.dt.bfloat16

ACT_FUNCS = [
    mybir.ActivationFunctionType.Sigmoid,  # i
    mybir.ActivationFunctionType.Sigmoid,  # f
    mybir.ActivationFunctionType.Sigmoid,  # o
    mybir.ActivationFunctionType.Tanh,     # g
]


def build_nc(T=T_FULL):
    n_grp = T // GRP
    nc = bacc.Bacc("TRN2", target_bir_lowering=False, debug=False,
                   num_devices=N_CORES)

    xt = nc.declare_dram_parameter("xt", [F, T * B], DBF16, isOutput=False)
    w_ih = nc.declare_dram_parameter("w_ih", [F, 512], DBF16, isOutput=False)
    w_hh = nc.declare_dram_parameter("w_hh", [H, 512], DBF16, isOutput=False)
    bias = nc.declare_dram_parameter("bias", [4, 128], FP32, isOutput=False)
    w_fc = nc.declare_dram_parameter("w_fc", [128, 2], DBF16, isOutput=False)
    ident = nc.declare_dram_parameter("ident", [128, 128], DBF16, isOutput=False)
    out = nc.declare_dram_parameter("out", [2, T * B], FP32, isOutput=True)

    # projection output: [grp][p][m][t][b] bf16
    pt = nc.dram_tensor("pt", [n_grp, 128, 4, GRP, B], DBF16)

    with tile.TileContext(nc) as tc:
        with (
            tc.tile_pool(name="wpool", bufs=1) as wpool,
            tc.tile_pool(name="xpool", bufs=32) as xpool,
            tc.tile_pool(name="pa_ps", bufs=2, space="PSUM") as pa_ps,
            tc.tile_pool(name="pa_out", bufs=3) as pa_out,
            tc.tile_pool(name="ptpool", bufs=2) as ptpool,
            tc.tile_pool(name="g_ps", bufs=2, space="PSUM") as g_ps,
            tc.tile_pool(name="act", bufs=2) as actp,
            tc.tile_pool(name="cell", bufs=2) as cellp,
            tc.tile_pool(name="state", bufs=1) as statep,
            tc.tile_pool(name="ro_ps", bufs=2, space="PSUM") as ro_ps,
            tc.tile_pool(name="ro_sb", bufs=2) as ro_sbp,
            tc.tile_pool(name="dram", bufs=2, space="DRAM") as dramp,
        ):
            # ---- load weights/constants ----
            w_ih_sb = wpool.tile([128, KF * 512], DBF16, tag="w_ih")
            for k in range(KF):
                nc.sync.dma_start(w_ih_sb[:, k * 512:(k + 1) * 512],
                                  w_ih[k * 128:(k + 1) * 128, :])
            w_hh_sb = wpool.tile([128, KH * 512], DBF16, tag="w_hh")
            for k in range(KH):
                nc.sync.dma_start(w_hh_sb[:, k * 512:(k + 1) * 512],
                                  w_hh[k * 128:(k + 1) * 128, :])
            ident_sb = wpool.tile([128, 128], DBF16, tag="ident")
            nc.sync.dma_start(ident_sb[:], ident[:])
            wfc_sb = wpool.tile([128, 2], DBF16, tag="wfc")
            nc.sync.dma_start(wfc_sb[:], w_fc[:])
            bias_sb = wpool.tile([128, 4], FP32, tag="bias")
            for m in range(4):
                nc.sync.dma_start(bias_sb[:, m:m + 1], bias[m, :][:, None])

            # ---- phase A: P = x @ W_ih.T (transposed layout) ----
            for grp in range(n_grp):
                xt_tiles = []
                for k in range(KF):
                    xtile = xpool.tile([128, 512], DBF16, tag="xt",
                                       name="xtile")
                    nc.sync.dma_start(
                        xtile[:],
                        xt[k * 128:(k + 1) * 128,
                           grp * GRP * B:(grp + 1) * GRP * B])
                    xt_tiles.append(xtile)
                for m in range(4):
                    ps = pa_ps.tile([128, 512], FP32, tag="pa")
                    for k in range(KF):
                        nc.tensor.matmul(
                            ps[:],
                            w_ih_sb[:, k * 512 + m * 128:k * 512 + (m + 1) * 128],
                            xt_tiles[k][:],
                            start=(k == 0), stop=(k == KF - 1))
                    ob = pa_out.tile([128, 512], DBF16, tag="pa_out")
                    nc.scalar.activation(ob[:], ps[:],
                                         mybir.ActivationFunctionType.Copy)
                    # dst [p][t][b] for fixed m; contiguous (t,b) runs
                    nc.sync.dma_start(pt[grp, :, m, :, :], ob[:])

            # ---- phase B: recurrence ----
            hT_bufs = [statep.tile([128, KH * B], DBF16, tag=f"hT{par}",
                                   name=f"hT{par}")
                       for par in range(2)]
            hist = statep.tile([128, T * B], DBF16, tag="hist")
            for par in range(2):
                nc.gpsimd.memset(hT_bufs[par][:], 0.0)
            c_prev = cellp.tile([128, B], FP32, tag="c")
            nc.gpsimd.memset(c_prev[:], 0.0)

            pt_grp = None
            ro_tile = None
            for t in range(T):
                par = t % 2
                if t % GRP == 0:
                    pt_grp = ptpool.tile([128, 4 * GRP * B], DBF16, tag="ptg")
                    nc.sync.dma_start(pt_grp[:], pt[t // GRP])
                if t % GRP == 0:
                    ro_tile = ro_ps.tile([2, GRP * B], FP32, tag="ro")

                # gates PSUM tile: partitions = unit-in-chunk, free = (m, b)
                ps = g_ps.tile([128, 4 * B], FP32, tag="g")
                # prefill with P (single matmul covering all 4 gate chunks)
                pt_view = pt_grp[:].rearrange("p (m t b) -> p m t b",
                                              m=4, t=GRP, b=B)
                nc.tensor.matmul(ps[:], ident_sb[:],
                                 pt_view[:, :, t % GRP, :],
                                 start=True, stop=False)
                # recurrent accumulation
                hT_cur = hT_bufs[par]
                for k in range(KH):
                    for m in range(4):
                        nc.tensor.matmul(
                            ps[:, m * B:(m + 1) * B],
                            w_hh_sb[:, k * 512 + m * 128:k * 512 + (m + 1) * 128],
                            hT_cur[:, k * B:(k + 1) * B],
                            start=False, stop=(k == KH - 1))

                # activations (bias folded in)
                acts = []
                for m in range(4):
                    a = actp.tile([128, B], FP32, tag=f"a{m}")
                    nc.scalar.activation(a[:], ps[:, m * B:(m + 1) * B],
                                         ACT_FUNCS[m], bias=bias_sb[:, m:m + 1])
                    acts.append(a)
                # cell update
                fc = cellp.tile([128, B], FP32, tag="fc")
                nc.vector.tensor_mul(fc[:], acts[1][:], c_prev[:])
                ig = cellp.tile([128, B], FP32, tag="ig")
                nc.vector.tensor_mul(ig[:], acts[0][:], acts[3][:])
                c_new = cellp.tile([128, B], FP32, tag="c")
                nc.vector.tensor_add(c_new[:], fc[:], ig[:])
                tc_t = cellp.tile([128, B], FP32, tag="tc")
                nc.scalar.activation(tc_t[:], c_new[:],
                                     mybir.ActivationFunctionType.Tanh)
                hslot = hist[:, t * B:(t + 1) * B]
                nc.vector.tensor_mul(hslot, acts[2][:], tc_t[:])
                c_prev = c_new

                # readout: out_ps[2, t%GRP block] = wfc.T @ h
                nc.tensor.matmul(ro_tile[:, (t % GRP) * B:(t % GRP + 1) * B],
                                 wfc_sb[:], hslot,
                                 start=True, stop=True)
                if t % GRP == GRP - 1:
                    ro_out = ro_sbp.tile([2, GRP * B], FP32, tag="ro_out")
                    nc.scalar.activation(ro_out[:], ro_tile[:],
                                         mybir.ActivationFunctionType.Copy)
                    nc.sync.dma_start(
                        out[:, (t - GRP + 1) * B:(t + 1) * B], ro_out[:])

                # exchange h shards (skip after last step)
                if t < T - 1:
                    bin_t = dramp.tile([128, B], DBF16, tag="cc_in",
                                       name="cc_in")
                    bout_t = dramp.tile([N_CORES * 128, B], DBF16,
                                        tag="cc_out", name="cc_out",
                                        addr_space="Shared")
                    nc.sync.dma_start(bin_t[:], hslot)
                    nc.gpsimd.collective_compute(
                        "AllGather", mybir.AluOpType.bypass,
                        ins=[bin_t[:]], outs=[bout_t[:]],
                        replica_groups=[list(range(N_CORES))])
                    dst = hT_bufs[1 - par]
                    nc.sync.dma_start(
                        dst[:].rearrange("p (k b) -> p k b", k=KH),
                        bout_t[:].rearrange("(k p) b -> p k b", p=128))
    nc.compile()
    return nc


def make_in_maps(x, W_ih, W_hh, b_ih, b_hh, W_fc, T=T_FULL):
    """Per-core input shards. Gate-chunk order m = [i, f, o, g]."""
    xt = np.ascontiguousarray(
        x[:, :T, :].transpose(2, 1, 0).reshape(F, T * B)).astype(BF16)
    bsum = (b_ih + b_hh).astype(np.float32)
    eye = np.eye(128, dtype=np.float32).astype(BF16)
    in_maps = []
    for p in range(N_CORES):
        rows = np.concatenate([
            np.arange(p * 128, (p + 1) * 128),            # i
            np.arange(H + p * 128, H + (p + 1) * 128),    # f
            np.arange(3 * H + p * 128, 3 * H + (p + 1) * 128),  # o
            np.arange(2 * H + p * 128, 2 * H + (p + 1) * 128),  # g
        ])
        in_maps.append({
            "xt": xt,
            "w_ih": np.ascontiguousarray(W_ih[rows, :].T).astype(BF16),
            "w_hh": np.ascontiguousarray(W_hh[rows, :].T).astype(BF16),
            "bias": np.ascontiguousarray(bsum[rows].reshape(4, 128)),
            "w_fc": np.ascontiguousarray(
                W_fc[:, p * 128:(p + 1) * 128].T).astype(BF16),
            "ident": eye,
        })
    return in_maps


def postprocess(results, b_fc, T=T_FULL):
    acc = np.zeros((2, T * B), np.float32)
    for r in results:
        acc += r["out"]
    out = acc.reshape(2, T, B).transpose(2, 1, 0) + b_fc[None, None, :]
    return np.ascontiguousarray(out.astype(np.float32))


def kernel(x, W_ih, W_hh, b_ih, b_hh, W_fc, b_fc):
    nc = build_nc(T_FULL)
    in_maps = make_in_maps(x, W_ih, W_hh, b_ih, b_hh, W_fc, T_FULL)
    res = run_bass_kernel_spmd(nc, in_maps, core_ids=list(range(N_CORES)))
    return postprocess(res.results, b_fc, T_FULL)


# revision 9
# speedup vs baseline: 1.0899x; 1.0899x over previous
"""Trainium2 Bass kernel for nn_ASD: LSTMCell over T=512 steps + linear readout.

Sharding: the 4096 gate columns (= 1024 hidden units x 4 gates) are sharded
8 ways; core p owns hidden units [128p, 128(p+1)) and computes their i/f/o/g
gates, cell state and hidden state. Each step the 8 hidden-state shards are
exchanged (AllGather) so every core has the full h_t for the next step's
recurrent matmul. Everything on-chip runs "transposed": hidden/gate units on
SBUF partitions, batch on the free dim, so no per-step transposes are needed
and elementwise ops use all 128 lanes.

Phase A precomputes P = x @ W_ih.T (+0) for this core's gate columns at full
PE utilization (moving free dim 512); phase B consumes P per step, adds the
recurrent term h_{t-1} @ W_hh.T via PSUM accumulation, applies the LSTM cell,
exchanges h shards, and accumulates the tiny readout matmul.

Compute dtype bf16 (PSUM accumulation fp32, cell state fp32).
"""
import sys
import numpy as np
import ml_dtypes

for _p in ("/opt/trn_rl_repo", "/root/.axon_site/_ro/trn_rl_repo"):
    if _p not in sys.path:
        sys.path.append(_p)

import concourse.bass as bass
import concourse.bacc as bacc
import concourse.mybir as mybir
import concourse.tile as tile
from concourse.bass_utils import run_bass_kernel_spmd

BF16 = ml_dtypes.bfloat16
N_CORES = 8
B = 32          # batch
F = 2048        # input features
H = 1024        # hidden
T_FULL = 512    # sequence length
KF = F // 128   # 16 feature chunks
KH = H // 128   # 8 hidden chunks
GRP = 16        # timesteps per projection group
FP32 = mybir.dt.float32
DBF16 = mybir.dt.bfloat16

ACT_FUNCS = [
    mybir.ActivationFunctionType.Sigmoid,  # i
    mybir.ActivationFunctionType.Sigmoid,  # f
    mybir.ActivationFunctionType.Sigmoid,  # o
    mybir.ActivationFunctionType.Tanh,     # g
]


def build_nc(T=T_FULL, comm=True, phase_b=True):
    n_grp = T // GRP
    nc = bacc.Bacc("TRN2", target_bir_lowering=False, debug=False,
                   num_devices=N_CORES)

    xt = nc.declare_dram_parameter("xt", [F, T * B], DBF16, isOutput=False)
    w_ih = nc.declare_dram_parameter("w_ih", [F, 512], DBF16, isOutput=False)
    w_hh = nc.declare_dram_parameter("w_hh", [H, 512], DBF16, isOutput=False)
    bias = nc.declare_dram_parameter("bias", [4, 128], FP32, isOutput=False)
    w_fc = nc.declare_dram_parameter("w_fc", [128, 2], DBF16, isOutput=False)
    ident = nc.declare_dram_parameter("ident", [128, 128], DBF16, isOutput=False)
    out = nc.declare_dram_parameter("out", [2, T * B], FP32, isOutput=True)

    # projection output: [grp][p][m][t][b] bf16
    pt = nc.dram_tensor("pt", [n_grp, 128, 4, GRP, B], DBF16)

    with tile.TileContext(nc) as tc:
        with (
            tc.tile_pool(name="wpool", bufs=1) as wpool,
            tc.tile_pool(name="xpool", bufs=32) as xpool,
            tc.tile_pool(name="pa_ps", bufs=2, space="PSUM") as pa_ps,
            tc.tile_pool(name="pa_out", bufs=3) as pa_out,
            tc.tile_pool(name="ptpool", bufs=2) as ptpool,
            tc.tile_pool(name="g_ps", bufs=2, space="PSUM") as g_ps,
            tc.tile_pool(name="act", bufs=2) as actp,
            tc.tile_pool(name="cell", bufs=2) as cellp,
            tc.tile_pool(name="state", bufs=1) as statep,
            tc.tile_pool(name="ro_ps", bufs=2, space="PSUM") as ro_ps,
            tc.tile_pool(name="ro_sb", bufs=2) as ro_sbp,
            tc.tile_pool(name="dram", bufs=2, space="DRAM") as dramp,
        ):
            # ---- load weights/constants ----
            w_ih_sb = wpool.tile([128, KF * 512], DBF16, tag="w_ih")
            for k in range(KF):
                nc.sync.dma_start(w_ih_sb[:, k * 512:(k + 1) * 512],
                                  w_ih[k * 128:(k + 1) * 128, :])
            w_hh_sb = wpool.tile([128, KH * 512], DBF16, tag="w_hh")
            for k in range(KH):
                nc.sync.dma_start(w_hh_sb[:, k * 512:(k + 1) * 512],
                                  w_hh[k * 128:(k + 1) * 128, :])
            ident_sb = wpool.tile([128, 128], DBF16, tag="ident")
            nc.sync.dma_start(ident_sb[:], ident[:])
            wfc_sb = wpool.tile([128, 2], DBF16, tag="wfc")
            nc.sync.dma_start(wfc_sb[:], w_fc[:])
            bias_sb = wpool.tile([128, 4], FP32, tag="bias")
            for m in range(4):
                nc.sync.dma_start(bias_sb[:, m:m + 1], bias[m, :][:, None])

            # ---- phase A: P = x @ W_ih.T (transposed layout) ----
            for grp in range(n_grp):
                xt_tiles = []
                for k in range(KF):
                    xtile = xpool.tile([128, 512], DBF16, tag="xt",
                                       name="xtile")
                    nc.sync.dma_start(
                        xtile[:],
                        xt[k * 128:(k + 1) * 128,
                           grp * GRP * B:(grp + 1) * GRP * B])
                    xt_tiles.append(xtile)
                for m in range(4):
                    ps = pa_ps.tile([128, 512], FP32, tag="pa")
                    for k in range(KF):
                        nc.tensor.matmul(
                            ps[:],
                            w_ih_sb[:, k * 512 + m * 128:k * 512 + (m + 1) * 128],
                            xt_tiles[k][:],
                            start=(k == 0), stop=(k == KF - 1))
                    ob = pa_out.tile([128, 512], DBF16, tag="pa_out")
                    nc.scalar.activation(ob[:], ps[:],
                                         mybir.ActivationFunctionType.Copy)
                    # dst [p][t][b] for fixed m; contiguous (t,b) runs
                    nc.sync.dma_start(pt[grp, :, m, :, :], ob[:])

            # ---- phase B: recurrence ----
            if not phase_b:
                dummy = pa_out.tile([2, T * B], FP32, tag="dummy", name="dummy")
                nc.gpsimd.memset(dummy[:], 0.0)
                nc.sync.dma_start(out[:], dummy[:])
            hT_bufs = [statep.tile([128, KH * B], DBF16, tag=f"hT{par}",
                                   name=f"hT{par}")
                       for par in range(2)]
            hist = statep.tile([128, T * B], DBF16, tag="hist")
            for par in range(2):
                nc.gpsimd.memset(hT_bufs[par][:], 0.0)
            c_prev = cellp.tile([128, B], FP32, tag="c")
            nc.gpsimd.memset(c_prev[:], 0.0)

            pt_grp = None
            ro_tile = None
            for t in range(T if phase_b else 0):
                par = t % 2
                if t % GRP == 0:
                    pt_grp = ptpool.tile([128, 4 * GRP * B], DBF16, tag="ptg")
                    nc.sync.dma_start(pt_grp[:], pt[t // GRP])
                if t % GRP == 0:
                    ro_tile = ro_ps.tile([2, GRP * B], FP32, tag="ro")

                # gates PSUM tile: partitions = unit-in-chunk, free = (m, b)
                ps = g_ps.tile([128, 4 * B], FP32, tag="g")
                # prefill with P (single matmul covering all 4 gate chunks)
                pt_view = pt_grp[:].rearrange("p (m t b) -> p m t b",
                                              m=4, t=GRP, b=B)
                nc.tensor.matmul(ps[:], ident_sb[:],
                                 pt_view[:, :, t % GRP, :],
                                 start=True, stop=False)
                # recurrent accumulation
                hT_cur = hT_bufs[par]
                for k in range(KH):
                    for m in range(4):
                        nc.tensor.matmul(
                            ps[:, m * B:(m + 1) * B],
                            w_hh_sb[:, k * 512 + m * 128:k * 512 + (m + 1) * 128],
                            hT_cur[:, k * B:(k + 1) * B],
                            start=False, stop=(k == KH - 1))

                # activations (bias folded in)
                acts = []
                for m in range(4):
                    a = actp.tile([128, B], FP32, tag=f"a{m}")
                    nc.scalar.activation(a[:], ps[:, m * B:(m + 1) * B],
                                         ACT_FUNCS[m], bias=bias_sb[:, m:m + 1])
                    acts.append(a)
                # cell update
                fc = cellp.tile([128, B], FP32, tag="fc")
                nc.vector.tensor_mul(fc[:], acts[1][:], c_prev[:])
                ig = cellp.tile([128, B], FP32, tag="ig")
                nc.vector.tensor_mul(ig[:], acts[0][:], acts[3][:])
                c_new = cellp.tile([128, B], FP32, tag="c")
                nc.vector.tensor_add(c_new[:], fc[:], ig[:])
                tc_t = cellp.tile([128, B], FP32, tag="tc")
                nc.scalar.activation(tc_t[:], c_new[:],
                                     mybir.ActivationFunctionType.Tanh)
                hslot = hist[:, t * B:(t + 1) * B]
                nc.vector.tensor_mul(hslot, acts[2][:], tc_t[:])
                c_prev = c_new

                # readout: out_ps[2, t%GRP block] = wfc.T @ h
                nc.tensor.matmul(ro_tile[:, (t % GRP) * B:(t % GRP + 1) * B],
                                 wfc_sb[:], hslot,
                                 start=True, stop=True)
                if t % GRP == GRP - 1:
                    ro_out = ro_sbp.tile([2, GRP * B], FP32, tag="ro_out")
                    nc.scalar.activation(ro_out[:], ro_tile[:],
                                         mybir.ActivationFunctionType.Copy)
                    nc.sync.dma_start(
                        out[:, (t - GRP + 1) * B:(t + 1) * B], ro_out[:])

                # exchange h shards (skip after last step)
                if comm and t < T - 1:
                    bin_t = dramp.tile([128, B], DBF16, tag="cc_in",
                                       name="cc_in")
                    bout_t = dramp.tile([N_CORES * 128, B], DBF16,
                                        tag="cc_out", name="cc_out",
                                        addr_space="Shared")
                    nc.sync.dma_start(bin_t[:], hslot)
                    nc.gpsimd.collective_compute(
                        "AllGather", mybir.AluOpType.bypass,
                        ins=[bin_t[:]], outs=[bout_t[:]],
                        replica_groups=[list(range(N_CORES))])
                    dst = hT_bufs[1 - par]
                    nc.sync.dma_start(
                        dst[:].rearrange("p (k b) -> p k b", k=KH),
                        bout_t[:].rearrange("(k p) b -> p k b", p=128))
    nc.compile()
    return nc


def make_in_maps(x, W_ih, W_hh, b_ih, b_hh, W_fc, T=T_FULL):
    """Per-core input shards. Gate-chunk order m = [i, f, o, g]."""
    xt = np.ascontiguousarray(
        x[:, :T, :].transpose(2, 1, 0).reshape(F, T * B)).astype(BF16)
    bsum = (b_ih + b_hh).astype(np.float32)
    eye = np.eye(128, dtype=np.float32).astype(BF16)
    in_maps = []
    for p in range(N_CORES):
        rows = np.concatenate([
            np.arange(p * 128, (p + 1) * 128),            # i
            np.arange(H + p * 128, H + (p + 1) * 128),    # f
            np.arange(3 * H + p * 128, 3 * H + (p + 1) * 128),  # o
            np.arange(2 * H + p * 128, 2 * H + (p + 1) * 128),  # g
        ])
        in_maps.append({
            "xt": xt,
            "w_ih": np.ascontiguousarray(W_ih[rows, :].T).astype(BF16),
            "w_hh": np.ascontiguousarray(W_hh[rows, :].T).astype(BF16),
            "bias": np.ascontiguousarray(bsum[rows].reshape(4, 128)),
            "w_fc": np.ascontiguousarray(
                W_fc[:, p * 128:(p + 1) * 128].T).astype(BF16),
            "ident": eye,
        })
    return in_maps


def postprocess(results, b_fc, T=T_FULL):
    acc = np.zeros((2, T * B), np.float32)
    for r in results:
        acc += r["out"]
    out = acc.reshape(2, T, B).transpose(2, 1, 0) + b_fc[None, None, :]
    return np.ascontiguousarray(out.astype(np.float32))


def kernel(x, W_ih, W_hh, b_ih, b_hh, W_fc, b_fc):
    nc = build_nc(T_FULL)
    in_maps = make_in_maps(x, W_ih, W_hh, b_ih, b_hh, W_fc, T_FULL)
    res = run_bass_kernel_spmd(nc, in_maps, core_ids=list(range(N_CORES)))
    return postprocess(res.results, b_fc, T_FULL)
